# revision 28
# baseline (speedup 1.0000x reference)
"""
CastratedGAT Trainium2 kernel (8 NeuronCores, SPMD, full-I/O contract).

Algorithm
---------
Reference computes a single GATConv-like layer:
  h = (x @ W).reshape(N, H, C);  a_src = sum(h*att_src, -1);  a_dst likewise
  per edge (dst <- src):  alpha = leaky_relu(a_src[src] + a_dst[dst], 0.2)
  segment softmax over each dst's neighborhood (incl. self loop), dropout on p,
  out[dst] = sum p * h[src]  (+ self term), + bias.

Key identity: with ex = exp(alpha) (no max-subtraction needed -- alpha is O(1))
and denom = segment_sum(ex),
  out[d,h,:] = ( sum_e ex*dp*h[src] ) / denom[d,h].

Design (v2)
-----------
- Nodes range-partitioned over 8 cores (6250 each); edges bucketed by dst so
  the segment reduction is core-local.  Every core computes the full node
  table T1[n] = [h_cmajor(256) | a_src(8) | a_dst(8) | pad] (replicated
  matmul, 384 bf16 cols = 768 B rows -- dma_gather needs 256B-multiple rows).
- h is stored C-MAJOR (col = c*8 + head).  This makes every broadcast-by-head
  DVE op have a packed (stride-1) inner dim of 8 -> 2x/4x DVE perf modes.
  The head-major order is restored on the host after the run.
- Destination windows are a UNIFORM 128 dsts (49 windows/core), so the output
  write is a plain contiguous HWDGE DMA (no indirect scatter).
- Per super-chunk (one window), all incident edges live in K = KLO+KHI chunks
  of 128 slots; source rows are fetched with TWO dma_gather instructions
  (table split at S0 because gather indices are int16), ~1.2us SWDGE each
  instead of one ~1us indirect DMA per 128 rows.
- Per 128-edge chunk a 0/1 selection matrix S[e,d]=(dloc[e]==d) is built with
  one batched iota-compare; psum[128,264] += S_j.T @ [w*h | ex] does the
  segment scatter-add for numerator and denominator at once on the PE.
- a_dst reaches edges via S.T: adw[128,8] (window rows of a_dst, one indirect
  row-gather per window) is expanded per-edge with a tiny St @ adw matmul;
  the St copies and exp run on the (otherwise idle) ACT engine, leaky-relu
  stays on DVE (the ACT Lrelu ignores its slope argument on HW).
"""

import math
import os

import numpy as np

# problem constants (hardcoded per contract -- kernel.py is self-contained)
N = 50000
E = 800000
F_IN = 128
H = 8
C = 32
HC = H * C  # 256
NCORES = 8
NLOC = N // NCORES   # 6250
P = 128              # partitions / edges per chunk
WIN = 128            # dst window (uniform)
NSC = (NLOC + WIN - 1) // WIN         # 98 super-chunks
NOUT = NSC * WIN                      # 6272 padded output rows
GRP = 1024                            # phase-0 node group
NPAD = int(math.ceil(N / GRP)) * GRP  # 50176
NBLK = NPAD // P                      # 392 (all-node 128-blocks)
ABLK = NOUT // P                      # 49 local a_dst blocks
TROW = 384           # T1 row: h(256)+a_src(8)+a_dst(8)+pad (768B, 256B mult)
TCOL = 272           # written cols of a T1 row

LAST_EXEC_NS = None
LAST_RESULTS = None


# ---------------------------------------------------------------- host prep

def _pick_split(all_src_by_core_sc):
    """Choose table split S0 and chunk counts (KLO, KHI).

    S0 must satisfy S0 <= 32768 and NPAD - S0 <= 32768 (int16 gather
    indices).  Minimise total chunks K = KLO + KHI over a few candidates.
    """
    lo_min = NPAD - 32768
    cands = sorted({lo_min + (32768 - lo_min) * i // 24 for i in range(25)})
    best = None
    for s0 in cands:
        if not (lo_min <= s0 <= 32768):
            continue
        klo = khi = 1
        for srcs in all_src_by_core_sc:
            nlo = int((srcs < s0).sum())
            klo = max(klo, (nlo + P - 1) // P)
            khi = max(khi, (len(srcs) - nlo + P - 1) // P)
        if best is None or klo + khi < best[0]:
            best = (klo + khi, s0, klo, khi)
    _, s0, klo, khi = best
    return s0, klo, khi


def _host_prep(x, edge_index, dp_mask, dp_mask_self, W, att_src, att_dst, bias,
               n, e, ncores):
    import ml_dtypes
    bf16 = ml_dtypes.bfloat16

    dst = np.asarray(edge_index[0], dtype=np.int64)
    src = np.asarray(edge_index[1], dtype=np.int64)
    loops = np.arange(n, dtype=np.int64)
    all_dst = np.concatenate([dst, loops])
    all_src = np.concatenate([src, loops])
    all_dp = np.concatenate([np.asarray(dp_mask, np.float32),
                             np.asarray(dp_mask_self, np.float32)], axis=0)

    order = np.argsort(all_dst, kind="stable")
    all_dst = all_dst[order]
    all_src = all_src[order]
    all_dp = all_dp[order]

    core_lo = np.searchsorted(all_dst, np.arange(ncores) * NLOC)
    core_hi = np.searchsorted(all_dst, (np.arange(ncores) + 1) * NLOC)

    # per-(core, sc) source lists
    per_sc = []  # [(core, sc)] -> (srcs, dlocs, dps)
    for m in range(ncores):
        lo, hi = core_lo[m], core_hi[m]
        d = all_dst[lo:hi] - m * NLOC
        s = all_src[lo:hi]
        dp = all_dp[lo:hi]
        win = d // WIN
        wlo = np.searchsorted(win, np.arange(NSC))
        whi = np.searchsorted(win, np.arange(NSC) + 1)
        row = []
        for sc in range(NSC):
            a, b = wlo[sc], whi[sc]
            row.append((s[a:b], (d[a:b] - sc * WIN), dp[a:b]))
        per_sc.append(row)

    s0, klo, khi = _pick_split(
        [per_sc[m][sc][0] for m in range(ncores) for sc in range(NSC)])
    k = klo + khi

    # streams: [NSC, 128, 9K] bf16 = [idx-bits(8K) | fv(K) | dp(8K)]
    streams = []
    for m in range(ncores):
        st = np.zeros((NSC, P, 17 * k), dtype=np.int16)
        stv = st.view(bf16)
        for sc in range(NSC):
            srcs, dlocs, dps = per_sc[m][sc]
            nlo_mask = srcs < s0
            for half, (kk, base) in enumerate(((klo, 0), (khi, klo))):
                sel = ~nlo_mask if half else nlo_mask
                ss = srcs[sel]
                dd = dlocs[sel]
                pp = dps[sel]
                nn = len(ss)
                assert nn <= kk * P
                idx = np.zeros(kk * P, dtype=np.int16)
                idx[:nn] = (ss - (s0 if half else 0)).astype(np.int16)
                blk = idx.reshape(-1, 16).T            # [16, kk*8]
                st[sc, :, base * 8:(base + kk) * 8] = np.tile(blk, (8, 1))
                # fv / dp: slot i -> chunk j=i//128 (+base), partition p=i%128
                fv = np.full(kk * P, 255.0, dtype=np.float32)
                fv[:nn] = dd.astype(np.float32)
                fvw = fv.reshape(kk, P).T               # [128, kk]
                stv[sc, :, 8 * k + base:8 * k + base + kk] = fvw.astype(bf16)
                dpw = np.zeros((kk * P, 8), dtype=np.float32)
                dpw[:nn] = pp
                dpw = dpw.reshape(kk, P, 8).transpose(1, 0, 2).reshape(P, kk * 8)
                stv[sc, :, 9 * k + base * 8:9 * k + (base + kk) * 8] = \
                    dpw.astype(bf16)
        streams.append(st.view(bf16))

    # a_dst gather offsets: core m, window sc, partition p -> row m*NLOC+sc*64+p
    aidx = [
        (m * NLOC + np.arange(NOUT, dtype=np.int32)).clip(0, NPAD - 1)
        .reshape(NSC, WIN).T.copy()
        for m in range(ncores)
    ]

    # replicated params
    xT = np.zeros((F_IN, NPAD), dtype=bf16)
    xT[:, :n] = np.asarray(x, np.float32).T.astype(bf16)
    Wf = np.asarray(W, np.float32)                       # [128, 256] h-major
    # c-major reorder: W_cm[:, c*8+h] = W[:, h*32+c]
    Wcm = Wf.reshape(F_IN, H, C).transpose(0, 2, 1).reshape(F_IN, HC)
    A = np.zeros((HC, 2 * H), dtype=np.float32)
    for hd in range(H):
        A[hd * C:(hd + 1) * C, hd] = np.asarray(att_src, np.float32)[hd]
        A[hd * C:(hd + 1) * C, H + hd] = np.asarray(att_dst, np.float32)[hd]
    WA = (Wf @ A).astype(bf16)                           # [128, 16]
    Wcm = Wcm.astype(bf16)

    in_maps = []
    for m in range(ncores):
        in_maps.append({
            "xT": xT, "W": Wcm, "WA": WA,
            "stream": streams[m], "aidx": aidx[m],
        })
    params = dict(s0=s0, klo=klo, khi=khi)
    return in_maps, params


# ---------------------------------------------------------------- device side

def _build(params, reps=1):
    ablate = os.environ.get("GAT_ABLATE", "")
    import concourse.bass as bass
    import concourse.bacc as bacc
    import concourse.mybir as mybir
    from concourse.tile import TileContext

    f32 = mybir.dt.float32
    i16 = mybir.dt.int16
    i32 = mybir.dt.int32
    bf16 = mybir.dt.bfloat16

    s0 = params["s0"]
    klo = params["klo"]
    khi = params["khi"]
    k = klo + khi
    # st psum tiles: <=8 chunks (2KB bank) each
    kparts = []
    off = 0
    while off < k:
        kparts.append((off, min(8, k - off)))
        off += min(8, k - off)

    nc = bacc.Bacc(None, target_bir_lowering=False)
    xT = nc.dram_tensor("xT", [F_IN, NPAD], bf16, kind="ExternalInput")
    W = nc.dram_tensor("W", [F_IN, HC], bf16, kind="ExternalInput")
    WA = nc.dram_tensor("WA", [F_IN, 2 * H], bf16, kind="ExternalInput")
    stream = nc.dram_tensor("stream", [NSC, P, 17 * k], bf16,
                            kind="ExternalInput")
    aidx = nc.dram_tensor("aidx", [WIN, NSC], i32, kind="ExternalInput")
    out = nc.dram_tensor("out", [NOUT, HC], f32, kind="ExternalOutput")
    T1 = nc.dram_tensor("T1", [NPAD, TROW], bf16, kind="Internal")

    ngrp = NPAD // GRP

    with TileContext(nc) as tc:
        with (
            tc.tile_pool(name="const", bufs=1) as cpool,
            tc.tile_pool(name="xt", bufs=3) as xpool,
            tc.tile_pool(name="t1o", bufs=3) as t1pool,
            tc.tile_pool(name="ps0", bufs=2, space="PSUM") as ps0,
            tc.tile_pool(name="stream", bufs=5) as spool,
            tc.tile_pool(name="gath", bufs=4) as gpool,
            tc.tile_pool(name="work", bufs=3) as wpool,
            tc.tile_pool(name="outp", bufs=3) as opool,
            tc.tile_pool(name="ps1", bufs=2, space="PSUM") as ps1,
            tc.tile_pool(name="pst", bufs=1, space="PSUM") as pst,
            tc.tile_pool(name="psa", bufs=1, space="PSUM") as psa,
        ):
            # constants
            w_sb = cpool.tile([F_IN, HC], bf16)
            nc.sync.dma_start(out=w_sb[:], in_=W[:, :])
            wa_sb = cpool.tile([F_IN, 2 * H], bf16)
            nc.sync.dma_start(out=wa_sb[:], in_=WA[:, :])
            aidx_sb = cpool.tile([WIN, NSC], i32)
            nc.sync.dma_start(out=aidx_sb[:], in_=aidx[:, :])
            from concourse.masks import make_identity
            ident = cpool.tile([P, P], bf16)
            make_identity(nc, ident[:])
            iota_i = cpool.tile([P, WIN], i32)
            nc.gpsimd.iota(iota_i[:], pattern=[[1, WIN]], base=0,
                           channel_multiplier=0)
            iota_b = cpool.tile([P, WIN], bf16)
            nc.vector.tensor_copy(out=iota_b[:], in_=iota_i[:])

            for _rep in range(reps):
                # ---------- phase 0: T1[n] = [x@W_cm | x@WA | pad] ----------
                for g in range(ngrp):
                    xt = xpool.tile([F_IN, GRP], bf16, tag="xt")
                    nc.sync.dma_start(out=xt[:],
                                      in_=xT[:, g * GRP:(g + 1) * GRP])
                    t1b = t1pool.tile([P, (GRP // P) * TCOL], bf16, tag="t1b")
                    for s in range(GRP // P):
                        psum = ps0.tile([P, TCOL], f32, tag="p0")
                        lhsT = xt[:, s * P:(s + 1) * P]
                        nc.tensor.matmul(psum[:, 0:HC], lhsT, w_sb[:],
                                         start=True, stop=True)
                        nc.tensor.matmul(psum[:, HC:TCOL], lhsT, wa_sb[:],
                                         start=True, stop=True)
                        nc.any.tensor_copy(
                            out=t1b[:, s * TCOL:(s + 1) * TCOL], in_=psum[:])
                    dv = T1[g * GRP:(g + 1) * GRP, 0:TCOL].rearrange(
                        "(s p) c -> p s c", p=P)
                    nc.sync.dma_start(
                        out=dv,
                        in_=t1b[:].rearrange("p (s c) -> p s c", c=TCOL))

                # ---------- phase 1: edge aggregation -----------------------
                for sc in range(NSC if ablate != "phase0" else 0):
                    st_t = spool.tile([P, 17 * k], bf16, tag="st")
                    nc.sync.dma_start(out=st_t[:], in_=stream[sc, :, :])
                    adw = spool.tile([WIN, H], bf16, tag="adw")
                    nc.gpsimd.indirect_dma_start(
                        out=adw[:], out_offset=None, in_=T1[:, :],
                        in_offset=bass.IndirectOffsetOnAxis(
                            ap=aidx_sb[:, sc:sc + 1], axis=0),
                        element_offset=HC + H,
                    )
                    G = gpool.tile([P, k * TROW], bf16, tag="G")
                    if ablate == "nogather":
                        nc.vector.memset(G[:], 0.1)
                    else:
                        nc.gpsimd.dma_gather(
                            out_ap=G[:, 0:klo * TROW].rearrange(
                                "p (j e) -> p j e", e=TROW),
                            in_ap=T1[0:s0, :],
                            idxs_ap=st_t[:, 0:klo * 8].bitcast(i16),
                            num_idxs=klo * P, num_idxs_reg=klo * P,
                            elem_size=TROW, single_packet=False)
                        nc.gpsimd.dma_gather(
                            out_ap=G[:, klo * TROW:k * TROW].rearrange(
                                "p (j e) -> p j e", e=TROW),
                            in_ap=T1[s0:NPAD, :],
                            idxs_ap=st_t[:, klo * 8:k * 8].bitcast(i16),
                            num_idxs=khi * P, num_idxs_reg=khi * P,
                            elem_size=TROW, single_packet=False)

                    # S matrices (one batched compare) + transposed copies
                    S = wpool.tile([P, k * WIN], bf16, tag="S")
                    nc.vector.tensor_tensor(
                        out=S[:].rearrange("p (j d) -> p j d", d=WIN),
                        in0=iota_b[:].unsqueeze(1).to_broadcast([P, k, WIN]),
                        in1=st_t[:, 8 * k:9 * k].unsqueeze(2)
                        .to_broadcast([P, k, WIN]),
                        op=mybir.AluOpType.is_equal)
                    st_sb = wpool.tile([WIN, k * P], bf16, tag="stsb")
                    for pi, (joff, jn) in enumerate(kparts):
                        stp = pst.tile([WIN, jn * P], bf16, tag=f"stp{pi}")
                        for jj in range(jn):
                            j = joff + jj
                            nc.tensor.transpose(
                                out=stp[:, jj * P:(jj + 1) * P],
                                in_=S[:, j * WIN:(j + 1) * WIN],
                                identity=ident[:])
                        nc.scalar.activation(
                            out=st_sb[:, joff * P:(joff + jn) * P],
                            in_=stp[:],
                            func=mybir.ActivationFunctionType.Copy)

                    # alpha = a_dst[dst] + a_src[src]; leaky; exp (ACT)
                    alpha_ps = psa.tile([P, k * H], f32, tag="alps")
                    for j in range(k):
                        nc.tensor.matmul(
                            alpha_ps[:, j * H:(j + 1) * H],
                            st_sb[:, j * P:(j + 1) * P], adw[:],
                            start=True, stop=True)
                    Gv = G[:].rearrange("p (j e) -> p j e", e=TROW)
                    asrc = Gv[:, :, HC:HC + H]
                    alpha = wpool.tile([P, k * H], bf16, tag="alpha")
                    nc.vector.tensor_tensor(
                        out=alpha[:].rearrange("p (j h) -> p j h", h=H),
                        in0=alpha_ps[:].rearrange("p (j h) -> p j h", h=H),
                        in1=asrc, op=mybir.AluOpType.add)
                    lr = wpool.tile([P, k * H], bf16, tag="lr")
                    nc.vector.scalar_tensor_tensor(
                        out=lr[:], in0=alpha[:], scalar=0.2, in1=alpha[:],
                        op0=mybir.AluOpType.mult, op1=mybir.AluOpType.max)
                    ex = wpool.tile([P, k * H], bf16, tag="ex")
                    nc.scalar.activation(
                        out=ex[:], in_=lr[:],
                        func=mybir.ActivationFunctionType.Exp)

                    # dpex = ex * dp;  G.h *= dpex (bcast over c);  G.asrc = ex
                    dpex = wpool.tile([P, k * H], bf16, tag="dpex")
                    nc.vector.tensor_tensor(
                        out=dpex[:], in0=ex[:],
                        in1=st_t[:, 9 * k:17 * k],
                        op=mybir.AluOpType.mult)
                    nc.vector.tensor_tensor(
                        out=Gv[:, :, 0:HC].rearrange(
                            "p j (c h) -> p j c h", h=H),
                        in0=Gv[:, :, 0:HC].rearrange(
                            "p j (c h) -> p j c h", h=H),
                        in1=dpex[:].rearrange("p (j h) -> p j h", h=H)
                        .unsqueeze(2).to_broadcast([P, k, C, H]),
                        op=mybir.AluOpType.mult)
                    nc.scalar.activation(
                        out=Gv[:, :, HC:HC + H],
                        in_=ex[:].rearrange("p (j h) -> p j h", h=H),
                        func=mybir.ActivationFunctionType.Copy)

                    # segment scatter-add: psum[64, 264] += S_j.T @ rhs_j
                    psum = ps1.tile([WIN, HC + H], f32, tag="p1")
                    for j in range(k):
                        nc.tensor.matmul(
                            psum[:],
                            S[:, j * WIN:(j + 1) * WIN],
                            G[:, j * TROW:j * TROW + HC + H],
                            start=(j == 0), stop=(j == k - 1))

                    # epilogue: divide by denom, write window rows
                    rec = wpool.tile([WIN, H], f32, tag="rec")
                    nc.vector.reciprocal(out=rec[:], in_=psum[:, HC:HC + H])
                    outt = opool.tile([WIN, HC], f32, tag="outt")
                    nc.vector.tensor_tensor(
                        out=outt[:].rearrange("p (c h) -> p c h", h=H),
                        in0=psum[:, 0:HC].rearrange("p (c h) -> p c h", h=H),
                        in1=rec[:].unsqueeze(1).to_broadcast([WIN, C, H]),
                        op=mybir.AluOpType.mult)
                    nc.sync.dma_start(
                        out=out[sc * WIN:(sc + 1) * WIN, :], in_=outt[:])
    nc.finalize()
    return nc


# ---------------------------------------------------------------- entry point

def kernel(**inputs):
    global LAST_EXEC_NS, LAST_RESULTS
    from concourse.bass_utils import run_bass_kernel_spmd

    in_maps, params = _host_prep(
        inputs["x"], inputs["edge_index"], inputs["dp_mask"],
        inputs["dp_mask_self"], inputs["W"], inputs["att_src"],
        inputs["att_dst"], inputs["bias"], N, E, NCORES)

    nc = _build(params)
    trace = bool(int(os.environ.get("GAT_TRACE", "0")))
    res = run_bass_kernel_spmd(nc, in_maps, core_ids=list(range(NCORES)),
                               trace=trace)
    LAST_EXEC_NS = res.exec_time_ns
    LAST_RESULTS = res
    # assemble: rows 6250 of 6272 per core; c-major -> h-major; + bias
    outs = []
    bias = np.asarray(inputs["bias"], np.float32)
    for m in range(NCORES):
        o = res.results[m]["out"][:NLOC]                      # [6250, 256] cm
        o = o.reshape(NLOC, C, H).transpose(0, 2, 1).reshape(NLOC, HC)
        outs.append(o + bias)
    return np.concatenate(outs, axis=0).astype(np.float32)
